# revision 38
# baseline (speedup 1.0000x reference)
"""Distributed Trainium2 (Bass/Tile) kernel for AdaptiveGCNLayer.

Reference semantics (N=4096 nodes, C=512 channels):
    adj   = x @ W_adj @ x.T + I                      [N, N]
    adj   = d^-1/2 * adj * d^-1/2   (row sums d)     -- values then DISCARDED:
    A     = (adj != 0) with forced unit diagonal     (dense_to_sparse keeps only
                                                      the nonzero pattern)
    deg   = A.sum(1); dis = deg^-1/2 (0 if deg<=0)
    out   = (dis[:,None] * A * dis[None,:]) @ (x @ W_gcn) + b

Scaling rows/cols by nonzero (or NaN/inf) factors never changes the !=0
pattern, so A == (x @ W_adj @ x.T != 0) except on the measure-zero event of
an exactly-zero f32 entry; the first normalization is therefore not
materialized, and the adjacency can be computed at any precision (fp8 here)
since only its zero pattern survives.

Sharding (8 cores, 1-D node partition, R=512 rows each): core i computes its
adjacency block in TRANSPOSED layout adjT [N, R] (directly usable as the
stationary operand of the final aggregation), masks it to {0,1} bf16, and
reduces mask -> deg for its rows (ones-matmul chain on the TensorEngine).

Collective structure: this environment has a ~10-100us (run-varying)
rank-dispatch skew — the mesh for ANY collective begins only once the
LAST core reaches its trigger (plus a constant ~11us CC start latency),
so the measured core-0 span is
  skew + slowest-core time-to-trigger + mesh + post-collective work.
The only collectives are TWO 8KB deg AllGathers, one per half of this
rank's row block: the first fires ~16us before the full deg would be
ready, and phase 3's contraction is split the same way (global tiles
whose dis arrived with exchange-a start aggregating while exchange-b is
still in flight).  y = x @ W_gcn is computed locally IN FULL on every
core (~30us of redundant TensorE work) inside the otherwise-dead wait
window — redundant FLOPs are free there, wire time is not.
Critical path: loads -> xwT -> adjacency+mask+deg half-a -> exchange-a
-> dis-a -> aggregation part 1 (with exchange-b/dis-b hidden under it)
-> aggregation part 2.

Scheduling notes (hard-won from traces):
  - ~7.3us fixed engine-bringup preamble before any user work
  - each DMA ring delivers ~100GB/s serialized in ring order, so ring
    POSITION is arrival time; the critical bytes (1a operands, then xT8)
    sit at the FRONT of the sync+scalar rings, split between them; the
    scalar ring stays short because the ACT engine's compute (and its
    activation-table load DMA) queue behind that ring's descriptors
  - xT8 is loaded in half-column chunks so adjacency tiles 0-15 can
    start ~2.5us before the full tensor lands
  - the Tile scheduler reorders per-engine instructions by READINESS,
    so program order alone cannot keep y-phase DVE/ACT work (casts) out
    of the mask window; the y casts are therefore a multiply by gate1, a
    1.0 DERIVED from the deg readback (is_gt(deg,0.5)), which data-gates
    them behind the collective trigger; the y matmuls themselves run
    free as tensor filler during the wait
  - the deg ones-matmul chain is interleaved at LAG 2 (deg for tile t-2
    issued after tile t's adjacency): at lag 0 it head-of-line-blocks
    the in-order tensor queue on every mask (~0.6us idle per tile and a
    PE p-state reset); fully deferred, the PE stalls on the mask pace
    mid-phase and never ramps — at lag 2 the chain is pure p-state
    filler and phase-2 matmuls run ~2x faster (measured 167ns vs 280+)
  - gpsimd's ring carries ONLY the deg bounce + AllGather trigger; the
    post-collective readbacks ride sync (gpsimd completion signaling
    adds ~5us); a short warm-up matmul burst gated on the gathered deg
    re-ramps the PE p-state before phase 3
  - mask computation is split DVE(not_equal) 2 : ACT(sign^2) 1, keeping
    both engines under the phase-2 wall; adjacency runs fp8 DoubleRow
  - the deg payload is written row-linear; readbacks pay the strided
    transpose (16KB, ~1us)
  - the bias enters through a rank-1 matmul sqrt(deg_r) (x) bias folded
    into the aggregation PSUM (cancels the later dis_r row scaling)
"""

import numpy as np

from concourse import bacc, mybir, tile
from concourse.bass_utils import run_bass_kernel_spmd

N_CORES = 8
N = 4096               # nodes
C = 512                # channels (C_IN == C_OUT)
R = N // N_CORES       # 512 rows per core
P = 128                # SBUF partitions
KT = C // P            # 4 contraction tiles
NT = N // P            # 32 node tiles
MT = R // P            # 4 row tiles per core
H = N // 2             # xT8 half-chunk columns

F32 = mybir.dt.float32
BF16 = mybir.dt.bfloat16
F8 = mybir.dt.float8e4
BF = mybir.dt.np(BF16)
F8NP = mybir.dt.np(F8)
DR = mybir.MatmulPerfMode.DoubleRow

_cache = {}


def _build():
    nc = bacc.Bacc("TRN2", target_bir_lowering=False, debug=False,
                   num_devices=N_CORES)

    xT8 = nc.dram_tensor("xT8", [C, N], F8, kind="ExternalInput")      # x^T, full
    xTs8 = nc.dram_tensor("xTs8", [C, R], F8, kind="ExternalInput")    # own cols
    adjW8 = nc.dram_tensor("adjW8", [C, C], F8, kind="ExternalInput")
    xTbf = nc.dram_tensor("xTbf", [C, N], BF16, kind="ExternalInput")  # x^T, full
    gcnW = nc.dram_tensor("gcnW", [C, C], BF16, kind="ExternalInput")
    bias = nc.dram_tensor("bias", [1, C], BF16, kind="ExternalInput")
    out = nc.dram_tensor("out", [R, C], F32, kind="ExternalOutput")

    rg = [list(range(N_CORES))]

    with tile.TileContext(nc) as tc:
        with (
            tc.tile_pool(name="sb", bufs=1) as sb,
            tc.tile_pool(name="sbo", bufs=2) as sbo,
            tc.tile_pool(name="dram", bufs=1, space="DRAM") as dram,
            tc.tile_pool(name="ps_a", bufs=2, space="PSUM") as ps_a,
            tc.tile_pool(name="ps_adj", bufs=3, space="PSUM") as ps_adj,
            tc.tile_pool(name="ps_deg", bufs=1, space="PSUM") as ps_deg,
            tc.tile_pool(name="ps_fin", bufs=2, space="PSUM") as ps_fin,
        ):
            # ---- SBUF tiles -------------------------------------------------
            adjW8_sb = sb.tile([P, KT, C], F8, name="adjW8_sb", tag="adjW8_sb")
            xTs8_sb = sb.tile([P, KT, R], F8, name="xTs8_sb", tag="xTs8_sb")
            xT8_sb = sb.tile([P, KT, N], F8, name="xT8_sb", tag="xT8_sb")
            xTbf_sb = [sb.tile([P, N], BF16, name=f"xTbf{k}", tag=f"xTbf{k}") for k in range(KT)]
            gcnW_sb = [sb.tile([P, C], BF16, name=f"gcnW{k}", tag=f"gcnW{k}") for k in range(KT)]
            bias_sb = sb.tile([1, C], BF16, name="bias_sb", tag="bias_sb")
            ones_col = sb.tile([P, 1], BF16, name="ones_col", tag="ones_col")
            ones_f32 = sb.tile([P, 1], F32, name="ones_f32", tag="ones_f32")
            scr = sb.tile([1, 8], F32, name="scr", tag="scr")

            # ---- input loads: ring position == arrival time ---------------
            # The ACT engine's compute shares its sequencer with the scalar
            # DMA ring, so the scalar ring carries ONLY the critical bytes
            # (1a operands + xT8) plus two ring-late xTbf slices; everything
            # else rides sync.
            nc.sync.dma_start(adjW8_sb[:, 0, :], adjW8[0:P, :])
            nc.sync.dma_start(adjW8_sb[:, 1, :], adjW8[P:2 * P, :])
            nc.sync.dma_start(xTs8_sb[:, 0, :], xTs8[0:P, :])
            nc.sync.dma_start(xTs8_sb[:, 1, :], xTs8[P:2 * P, :])
            nc.scalar.dma_start(adjW8_sb[:, 2, :], adjW8[2 * P:3 * P, :])
            nc.scalar.dma_start(adjW8_sb[:, 3, :], adjW8[3 * P:4 * P, :])
            nc.scalar.dma_start(xTs8_sb[:, 2, :], xTs8[2 * P:3 * P, :])
            nc.scalar.dma_start(xTs8_sb[:, 3, :], xTs8[3 * P:4 * P, :])
            for h in range(2):
                nc.sync.dma_start(xT8_sb[:, 0, H * h:H * (h + 1)], xT8[0:P, H * h:H * (h + 1)])
                nc.sync.dma_start(xT8_sb[:, 1, H * h:H * (h + 1)], xT8[P:2 * P, H * h:H * (h + 1)])
                nc.scalar.dma_start(xT8_sb[:, 2, H * h:H * (h + 1)], xT8[2 * P:3 * P, H * h:H * (h + 1)])
                nc.scalar.dma_start(xT8_sb[:, 3, H * h:H * (h + 1)], xT8[3 * P:4 * P, H * h:H * (h + 1)])
            nc.sync.dma_start(bias_sb[:, :], bias[:, :])
            for k in range(KT):
                nc.sync.dma_start(gcnW_sb[k][:, :], gcnW[P * k:P * (k + 1), :])
            # y stationary ring-LATE: y matmuls start only as these land, so
            # their casts cannot crowd the mask window; the y accumulation
            # consumes the k-slices in arrival order (1,0,3,2 across rings)
            nc.sync.dma_start(xTbf_sb[0][:, :], xTbf[0:P, :])
            nc.scalar.dma_start(xTbf_sb[1][:, :], xTbf[P:2 * P, :])
            nc.sync.dma_start(xTbf_sb[2][:, :], xTbf[2 * P:3 * P, :])
            nc.scalar.dma_start(xTbf_sb[3][:, :], xTbf[3 * P:4 * P, :])

            nc.vector.memset(ones_col[:, :], 1.0)
            nc.vector.memset(ones_f32[:, :], 1.0)
            # preload DVE reciprocal / ACT sqrt+sign lookup tables off the
            # critical path (first use otherwise costs ~1.3us each)
            nc.vector.memset(scr[:, 0:4], 4.0)
            nc.vector.reciprocal(scr[:, 4:8], scr[:, 0:4])
            nc.scalar.sqrt(scr[:, 4:8], scr[:, 0:4])
            nc.scalar.sign(scr[:, 4:8], scr[:, 0:4])

            # ---- phase 1a: xwT[j, r] = sum_c W_adj[c, j] x^T[c, r]  (fp8 DR)
            xwT8_sb = sb.tile([P, KT, R], F8, name="xwT8_sb", tag="xwT8_sb")
            for j in range(KT):
                pa = ps_a.tile([P, R], F32, name=f"psa{j}", tag="psa")
                for k in range(0, KT, 2):
                    nc.tensor.matmul(pa[:, :],
                                     adjW8_sb[:, k:k + 2, P * j:P * (j + 1)],
                                     xTs8_sb[:, k:k + 2, :],
                                     start=(k == 0), stop=(k == KT - 2),
                                     perf_mode=DR)
                nc.vector.tensor_copy(xwT8_sb[:, j, :], pa[:, :])

            # ---- phase 2: adjT tiles (fp8 DR), mask (bf16), deg ------------
            # Row-split halves: rows [0,256) and [256,512) of this rank's
            # adjacency block are processed as separate pipelines, each with
            # its own deg chain + 8KB AllGather.  Half-a's exchange fires
            # ~17us before the full deg would be ready, and phase 3's
            # contraction splits the same way (global tiles tau with
            # tau%4 < 2 have their dis after exchange-a), pipelining the
            # collectives against the aggregation.
            R2 = R // 2
            mask_sb = [sb.tile([P, R], BF16, name=f"mask{t}", tag=f"mask{t}") for t in range(NT)]
            deg_own_h = []
            invdis_h = []
            degb_in_h = []
            degb_out_h = []
            def deg_mm(pdeg, h, t, start, stop):
                nc.tensor.matmul(pdeg[:, :], ones_col[:, :],
                                 mask_sb[t][:, R2 * h:R2 * (h + 1)],
                                 start=start, stop=stop)

            for h in range(2):
                pdeg = ps_deg.tile([1, R2], F32, name=f"pdeg{h}", tag="pdeg")
                for t in range(NT):
                    pt = ps_adj.tile([P, R2], F32, name=f"psadj{h}_{t}", tag="psadj")
                    for k in range(0, KT, 2):
                        nc.tensor.matmul(pt[:, :],
                                         xT8_sb[:, k:k + 2, P * t:P * (t + 1)],
                                         xwT8_sb[:, k:k + 2, R2 * h:R2 * (h + 1)],
                                         start=(k == 0), stop=(k == KT - 2),
                                         perf_mode=DR)
                    mv = mask_sb[t][:, R2 * h:R2 * (h + 1)]
                    # mask split DVE (not_equal) 2 : ACT (sign^2) 1
                    if t % 3 == 2:
                        nc.scalar.sign(mv, pt[:, :])
                        nc.scalar.square(mv, mv)
                    else:
                        nc.vector.tensor_scalar(mv, pt[:, :], 0.0, None,
                                                mybir.AluOpType.not_equal)
                    # deg chain interleaved at LAG 2: mask[t-2] is long done
                    # when the tensor reaches this, so it is pure filler that
                    # keeps the PE's p-state ramped (a 0-lag interleave
                    # head-of-line-blocks the queue on every mask; a fully
                    # deferred chain leaves the PE stalling mid-phase)
                    if t >= 2:
                        deg_mm(pdeg, h, t - 2, start=(t == 2), stop=False)
                deg_mm(pdeg, h, NT - 2, start=False, stop=False)
                deg_mm(pdeg, h, NT - 1, start=False, stop=True)

                dg = sb.tile([1, R2], F32, name=f"deg_own{h}", tag=f"deg_own{h}")
                nc.vector.tensor_copy(dg[:, :], pdeg[:, :])
                deg_own_h.append(dg)
                # sqrt(deg) row-vector: cancels dis_r row scaling for the bias
                iv = sb.tile([1, R2], BF16, name=f"invdis{h}", tag=f"invdis{h}")
                nc.scalar.sqrt(iv[:, :], dg[:, :])
                invdis_h.append(iv)

                dbi = dram.tile([R2], F32, name=f"degb_in{h}", tag=f"degb_in{h}")
                dbo = dram.tile([N // 2], F32, addr_space="Shared",
                                name=f"degb_out{h}", tag=f"degb_out{h}")
                # row-linear payload: degb_in[r] = deg(own row R2*h + r)
                nc.gpsimd.dma_start(dbi.rearrange("(m p) -> m p", p=P), dg[:, :])
                nc.gpsimd.collective_compute(
                    "AllGather", mybir.AluOpType.bypass, replica_groups=rg,
                    ins=[dbi.opt()], outs=[dbo.opt()])
                degb_in_h.append(dbi)
                degb_out_h.append(dbo)

            # readbacks ride SYNC (clear by then, and its completion
            # signaling is ~5us faster than gpsimd's)
            deg_ownp_h = []
            deg_glob_h = []
            dis_own_h = []
            for h in range(2):
                dop = sb.tile([P, 2], F32, name=f"deg_ownp{h}", tag=f"deg_ownp{h}")
                nc.sync.dma_start(dop[:, :], degb_in_h[h].rearrange("(m p) -> p m", p=P))
                deg_ownp_h.append(dop)
                dgl = sb.tile([P, NT // 2], F32, name=f"deg_glob{h}", tag=f"deg_glob{h}")
                nc.sync.dma_start(
                    dgl[:, :].rearrange("p (i m) -> p i m", i=N_CORES),
                    degb_out_h[h].rearrange("(i m p) -> p i m", i=N_CORES, p=P))
                deg_glob_h.append(dgl)
                dso = sb.tile([P, 2], F32, name=f"dis_own{h}", tag=f"dis_own{h}")
                nc.vector.reciprocal(dso[:, :], dop[:, :])
                nc.scalar.sqrt(dso[:, :], dso[:, :])
                dis_own_h.append(dso)

            # cast gate DERIVED from deg data (deg >= 1 so this is 1.0): the
            # y casts multiply by it, which pins them after the first deg
            # half — a bare memset would be hoisted by the readiness
            # scheduler and the casts would crowd the mask window on DVE/ACT
            gate1 = sb.tile([P, 1], F32, name="gate1", tag="gate1")
            nc.vector.tensor_scalar(gate1[:, :], deg_ownp_h[0][:, 0:1], 0.5, None,
                                    mybir.AluOpType.is_gt)

            # fence: rotate the ps_a pool with tiny matmuls gated on the
            # FIRST deg half, so y matmuls cannot dilute half-a's adjacency
            # production on the tensor queue (the readiness scheduler has no
            # notion of criticality; without this, half-a's mask window runs
            # ~24us instead of ~16us).  y still fills the half-b window.
            for s in range(2):
                pd = ps_a.tile([1, 16], F32, name=f"yfence{s}", tag="psa")
                nc.tensor.matmul(pd[:, :], invdis_h[0][:, 0:1],
                                 invdis_h[0][:, 0:16], start=True, stop=True)

            # ---- full y = x @ W_gcn for ALL nodes (local, no collective) ---
            # y matmuls fill the half-b + exchange windows (tensor p-state
            # filler); the CASTS are gated on gate1 (post-deg-a) so DVE/ACT
            # stay exclusive to masks until the trigger fires.
            # k-order (1,0,3,2) matches the xTbf arrival order across rings
            YK = (1, 0, 3, 2)
            y_sb = [sb.tile([P, C], BF16, name=f"y{t}", tag=f"y{t}") for t in range(NT)]
            for t in range(NT):
                pa = ps_a.tile([P, C], F32, name=f"psy{t}", tag="psa")
                for ki, k in enumerate(YK):
                    nc.tensor.matmul(pa[:, :],
                                     xTbf_sb[k][:, P * t:P * (t + 1)],
                                     gcnW_sb[k][:, :],
                                     start=(ki == 0), stop=(ki == KT - 1))
                if t % 3 == 2:
                    nc.scalar.mul(y_sb[t][:, :], pa[:, :], gate1[:, 0:1])
                else:
                    nc.vector.tensor_scalar(y_sb[t][:, :], pa[:, :],
                                            gate1[:, 0:1], None,
                                            mybir.AluOpType.mult)

            # dis = deg^-1/2 (global, per half, post-AllGather)
            dis_glob_h = []
            for h in range(2):
                dgl = sb.tile([P, NT // 2], F32, name=f"dis_glob{h}", tag=f"dis_glob{h}")
                nc.vector.reciprocal(dgl[:, :], deg_glob_h[h][:, :])
                nc.scalar.sqrt(dgl[:, :], dgl[:, :])
                dis_glob_h.append(dgl)

            # tensor warm-up gated on the first gathered deg half: ramps the
            # PE p-state during the dis/scale latency so phase 3 starts hot
            psw = ps_deg.tile([1, R], F32, name="psw", tag="pdeg")
            nc.tensor.matmul(psw[:, 0:NT // 2], ones_f32[:, :], deg_glob_h[0][:, :],
                             start=True, stop=True)
            for w in range(6):
                nc.tensor.matmul(psw[:, :], ones_col[:, :], mask_sb[NT - 1][:, :],
                                 start=True, stop=True)

            # ---- phase 3: y *= dis; out_rows = dis_r * (A @ y) + b ----------
            # global tile tau = 4i+m holds nodes owned by rank i, row block
            # m: its dis lives in half h = m//2 at column 2i + (m - 2h)
            def dis_col(tau):
                i, m = tau // 4, tau % 4
                return m // 2, 2 * i + (m % 2)

            for t in range(NT):
                h, col = dis_col(t)
                if t % 3 == 2:
                    nc.scalar.mul(y_sb[t][:, :], y_sb[t][:, :],
                                  dis_glob_h[h][:, col:col + 1])
                else:
                    nc.vector.tensor_scalar(y_sb[t][:, :], y_sb[t][:, :],
                                            dis_glob_h[h][:, col:col + 1], None,
                                            mybir.AluOpType.mult)

            # m-outer; within each m-chain the contraction tiles whose dis
            # arrived with exchange-a run first, the rest (and the bias) run
            # once exchange-b has landed — the b-exchange latency hides
            # behind the first half of the aggregation
            p1 = [t for t in range(NT) if t % 4 < 2]
            p2 = [t for t in range(NT) if t % 4 >= 2]
            for m in range(MT):
                pf = ps_fin.tile([P, C], F32, name=f"psf{m}", tag="psf")
                for ti, t in enumerate(p1):
                    nc.tensor.matmul(pf[:, :],
                                     mask_sb[t][:, P * m:P * (m + 1)],
                                     y_sb[t][:, :],
                                     start=(ti == 0), stop=False)
                for t in p2:
                    nc.tensor.matmul(pf[:, :],
                                     mask_sb[t][:, P * m:P * (m + 1)],
                                     y_sb[t][:, :],
                                     start=False, stop=False)
                # += sqrt(deg_r) (x) bias  — cancels against the dis_r scaling
                mh, mo = m // 2, m % 2
                nc.tensor.matmul(pf[:, :],
                                 invdis_h[mh][:, P * mo:P * (mo + 1)],
                                 bias_sb[:, :],
                                 start=False, stop=True)
                ot = sbo.tile([P, C], F32, name=f"outt{m}", tag="outt")
                nc.vector.tensor_scalar(ot[:, :], pf[:, :],
                                        dis_own_h[mh][:, mo:mo + 1],
                                        None, mybir.AluOpType.mult)
                eng = nc.sync if m % 2 == 0 else nc.scalar
                eng.dma_start(out[P * m:P * (m + 1), :], ot[:, :])

    nc.compile()
    return nc


def _get_nc():
    if "nc" not in _cache:
        _cache["nc"] = _build()
    return _cache["nc"]


def _run(inputs, trace=False, trace_cores=None):
    x = np.asarray(inputs["x"], dtype=np.float32)
    adj_weight = np.asarray(inputs["adj_weight"], dtype=np.float32)
    gcn_weight = np.asarray(inputs["gcn_weight"], dtype=np.float32)
    gcn_bias = np.asarray(inputs["gcn_bias"], dtype=np.float32)

    xT = np.ascontiguousarray(x.T)                     # [C, N] f32
    xT8 = xT.astype(F8NP)
    xTbf = xT.astype(BF)
    adjW8 = adj_weight.astype(F8NP)
    gcnW = gcn_weight.astype(BF)
    bias_bf = gcn_bias.reshape(1, C).astype(BF)

    in_maps = []
    for i in range(N_CORES):
        in_maps.append({
            "xT8": xT8,
            "xTs8": np.ascontiguousarray(xT8[:, R * i:R * (i + 1)]),
            "adjW8": adjW8,
            "xTbf": xTbf,
            "gcnW": gcnW,
            "bias": bias_bf,
        })

    nc = _get_nc()
    res = run_bass_kernel_spmd(nc, in_maps, core_ids=list(range(N_CORES)),
                               trace=trace, trace_cores=trace_cores)
    full = np.concatenate([res.results[i]["out"] for i in range(N_CORES)], axis=0)
    return full, res


def kernel(**inputs):
    full, _ = _run(inputs, trace=False)
    return full
